# revision 100
# baseline (speedup 1.0000x reference)
"""Bass/Trainium2 kernel for nn_BiGRIL (gnn_message_passing).

Algebraic structure (h == 0, C == 1):
  u    = (x - bfs)*m                    (host-computed input prep)
  z    = W0*x1 + W1*m + b_in            (rank-2 in channels)
  zg   = A^T z  ->  W0*ug + W1*mg + b_in'*cg   with ug = A^T u, mg = A^T m
  v    = PA6^T [1,u,m,ug,mg,cg]         (K=11 matmul; bias via ones-row)
  o    = PReLU(v)                       (ACT Prelu, psum->fp16)
  w    = PB^T o                         (pass B; PB = wro1 (x) W_o1 rank-1)
  rr   = relu(w + bias_f)               (DVE TS over [128,1024] couples)
  out  = W_o2 . rr + b_o2               (pass C; +b_o2 via final copy bias)

Per-pair pipeline (chunk pair g = lanes g, g+64; 512 positions each):
A(p) / B(p-2) / C(p-4) like the reference schedule, with:
 - K=11 moving tiles (shared ones-row carries bias_v; no dead rows, no
   memsets) whose u/m/cg rows stream position-major straight from HBM
 - pass B couples write a [128,1024] 2-bank psum tile; ONE wide DVE
   tensor-scalar relu per couple
 - pass C packs its 2 output rows per pair into ONE accumulated psum
   bank (rows 2g, 2g+1 via 16 stationary column-variants + tile_position
   + start=False accumulation), so the whole output drains with a single
   copy + a single 128KB DMA
 - adj grouped mt-major so G-tiles start as stripes land; G psums borrow
   the vps pool

Sharding: data-parallel over batch (B=8 -> 8 cores), no collectives.
"""

import numpy as np
import sys

sys.path.insert(0, "/opt/trn_rl_repo")

B, C, N, T = 8, 1, 1024, 64
H = 64
NT = N * T          # 65536 per-core output elements
CHUNK = 512
NPAIR = 64          # pairs (g, g+64); chunk c=g is lane0, c=g+64 lane1
HALF = NT // 2      # 32768: lane1 position offset
BLK = 4096          # ma tile columns (64 nodes x 64 steps)
NBLK = 8            # blocks per lane

_CACHE = {}


def _fold_weights(W_fs, b_fs, W_in, b_in, W_gc, b_gc, W_lo, b_lo, prelu_a,
                  W_ro, b_ro, W_o1, b_o1, W_o2, b_o2, adj):
    """Host-side weight folding in float64 for accuracy."""
    f8 = np.float64
    W_in, b_in = W_in.astype(f8), b_in.astype(f8)
    W_gc, b_gc = W_gc.astype(f8), b_gc.astype(f8)
    W_lo, b_lo = W_lo.astype(f8), b_lo.astype(f8)
    W_ro, b_ro = W_ro.astype(f8), b_ro.astype(f8)
    W_o1, b_o1 = W_o1.astype(f8), b_o1.astype(f8)
    W_o2, b_o2 = W_o2.astype(f8), b_o2.astype(f8)

    W0 = W_in[:, 0]           # x1 channel  [64]
    W1 = W_in[:, 1]           # mask channel [64]
    Wlo1 = W_lo[:, :H]
    M1 = Wlo1 @ W_gc[:, :H]
    M2 = Wlo1 @ W_gc[:, H:]
    b_fold = Wlo1 @ b_gc + b_lo

    b_in_p = b_in + W0 * float(b_fs[0])
    PA = np.stack([
        M1 @ W0,
        M1 @ W1,
        M2 @ W0,
        M2 @ W1,
        M2 @ b_in_p,
    ])                                     # [5, 64]  lhsT for pass A
    bias_v = M1 @ b_in_p + b_fold          # [64] -> ones-row of pass A

    w_ro1 = W_ro[0, :H]                    # [64]
    PB = np.outer(w_ro1, W_o1[:, 0])       # [64(h), 64(f)] lhsT for pass B
    bias_f = W_o1[:, 0] * b_ro[0] + b_o1   # [64]

    cg = adj.astype(f8).sum(axis=0)        # [N] column sums of adj
    cgrep = np.repeat(cg, T)               # [(n,t)] layout n*T + t

    a = float(prelu_a)
    assert 0.0 < a < 1.0

    # pass A stationary [128, 128]: row 0 = shared ones stream carrying
    # bias_v for BOTH lanes; rows 1-6 = HBM slab (u0,m0,cg0,u1,m1,cg1);
    # rows 7-10 = gathered (ug0,mg0,ug1,mg1).  Lane0 -> out cols 0:64,
    # lane1 -> cols 64:128.  Rows 11:128 are zero: K stays 128 because
    # K<128 matmuls pin the PE HAM activity gating at 4/8 (half rate).
    paA = np.zeros((128, 128))
    paA[0, 0:64] = bias_v
    paA[0, 64:128] = bias_v
    paA[1, 0:64] = PA[0]        # u0
    paA[2, 0:64] = PA[1]        # m0
    paA[3, 0:64] = PA[4]        # cg0
    paA[4, 64:128] = PA[0]      # u1
    paA[5, 64:128] = PA[1]      # m1
    paA[6, 64:128] = PA[4]      # cg1
    paA[7, 0:64] = PA[2]        # ug0
    paA[8, 0:64] = PA[3]        # mg0
    paA[9, 64:128] = PA[2]      # ug1
    paA[10, 64:128] = PA[3]     # mg1
    # pass B stationary: blockdiag(PB, PB) over packed prelu [128, 512]
    pbB = np.zeros((128, 128))
    pbB[0:64, 0:64] = PB
    pbB[64:128, 64:128] = PB
    # pass C stationaries: 16 variants [128, 32]; variant k places lane0 at
    # col 2k, lane1 at col 2k+1 -> accumulated po-bank row 32t + 2k + lane
    # for pair g with k = g%16, t = g//16
    pc = np.zeros((128, 16 * 32))
    for k in range(16):
        pc[0:64, 32 * k + 2 * k] = W_o2[0]
        pc[64:128, 32 * k + 2 * k + 1] = W_o2[0]

    h16 = np.float16
    fp = np.float32
    ch = np.zeros((128, 776), h16)
    ch[:, 0:128] = paA.astype(h16)
    ch[:, 128:256] = pbB.astype(h16)
    ch[:, 256:768] = pc.astype(h16)
    cf = np.zeros((128, 4), fp)
    cf[0:64, 0] = bias_f           # rr relu bias
    cf[64:128, 0] = bias_f
    cf[:, 1] = float(b_o2[0])      # final copy bias (all rows are outputs)
    # cols 2,3 stay zero: zero-scalar operands for DVE tensor_scalar ops
    ch[:, 768:776] = cf.view(h16)  # fp32 scalars bit-packed as fp16 cols

    return dict(
        consts_h=ch,
        cgrep=cgrep.astype(h16),
        prelu_a=a,
    )


def _build_program(prelu_a):
    import concourse.bass as bass
    import concourse.bacc as bacc
    import concourse.mybir as mybir
    import concourse.tile as tile

    dt = mybir.dt
    f32 = dt.float32
    h16 = dt.float16
    AF = mybir.ActivationFunctionType
    ALU = mybir.AluOpType

    nc = bacc.Bacc("TRN2", target_bir_lowering=False, debug=False,
                   num_devices=B)

    ones_d = nc.dram_tensor("ones", [4 * BLK], h16, kind="ExternalInput")
    zeros_d = nc.dram_tensor("zeros", [128, 4 * BLK], h16,
                             kind="ExternalInput")
    # per-block 6-row slab: (u0, m0, cg0, u1, m1, cg1) x 4096, ma rows 1:7
    rows_d = nc.dram_tensor("rows", [NBLK, 6, BLK], h16,
                            kind="ExternalInput")
    # packed [um | stripe0 | stripe4 | stripes 1,2,3 | stripes 5,6,7]
    adjs = nc.dram_tensor("adjs", [128, 9216], h16, kind="ExternalInput")
    # consts: fp16 cols 0:768 + 8 fp16 cols holding the 4 fp32 scalars
    ch_d = nc.dram_tensor("consts_h", [128, 776], h16, kind="ExternalInput")
    # row-major [128, 512] final tile; row 2g+lane = pair g, lane
    out_d = nc.dram_tensor("out", [NT], h16, kind="ExternalOutput")

    MOVA_BUFS = 4
    from contextlib import ExitStack
    with tile.TileContext(nc) as tc, ExitStack() as ctx:
        const = ctx.enter_context(tc.tile_pool(name="const", bufs=1))
        adjp = ctx.enter_context(tc.tile_pool(name="adjp", bufs=1))
        ump = ctx.enter_context(tc.tile_pool(name="ump", bufs=1))
        gxp = ctx.enter_context(tc.tile_pool(name="gxp", bufs=1))
        movap = ctx.enter_context(tc.tile_pool(name="movap", bufs=1))
        op_p = ctx.enter_context(tc.tile_pool(name="op", bufs=6))
        rrp = ctx.enter_context(tc.tile_pool(name="rrp", bufs=3))
        posbp = ctx.enter_context(tc.tile_pool(name="posbp", bufs=1))
        # PSUM budget (8 banks): vps 3x1 + wps 2x2 + pop 1
        vps = ctx.enter_context(tc.tile_pool(name="vps", bufs=3, space="PSUM"))
        wps = ctx.enter_context(tc.tile_pool(name="wps", bufs=2, space="PSUM"))
        pop = ctx.enter_context(tc.tile_pool(name="pop", bufs=1, space="PSUM"))

        # ---- single consolidated constant load (emitted after the
        # G-critical loads below; nothing needs consts before pair 0) ----
        ch_t = const.tile([128, 776], h16)
        pa_t = ch_t[:, 0:128]
        pb_t = ch_t[:, 128:256]
        pc_t = [ch_t[:, 256 + 32 * k:256 + 32 * (k + 1)] for k in range(16)]
        cf_t = ch_t[:, 768:776].bitcast(f32)   # [128, 4] fp32 view
        bf_t = cf_t[:, 0:1]
        bo2_t = cf_t[:, 1:2]
        zero_t = cf_t[:, 3:4]

        # ---- HAM warmup on a memset scratch: NO DMA dependency, so the
        # PE starts streaming (and ramping clock) at t~0 -------------------
        po_ps = pop.tile([128, 512], f32, tag="po", name="po")
        scr = const.tile([128, 128], h16)
        nc.gpsimd.memset(scr[:, :], 0.0)
        for wi in range(72):
            nc.tensor.matmul(po_ps[:, 0:128], scr[:], scr[:],
                             start=True, stop=True, skip_group_check=True)

        # G inputs ride the sync ring (parallel with the ma-chain on the
        # gpsimd ring).  adjt packs [um | stripe0 | stripe4 | 1,2,3 |
        # 5,6,7] so G0's critical data is ONE DMA.
        adjt = adjp.tile([128, 9216], h16, tag="adjt", name="adjt")
        # G0-critical halves land in parallel on both hw-DGE rings
        nc.sync.dma_start(out=adjt[:, 0:1024], in_=adjs[:, 0:1024])
        nc.scalar.dma_start(out=adjt[:, 1024:2048], in_=adjs[:, 1024:2048])
        nc.sync.dma_start(out=adjt[:, 2048:3072], in_=adjs[:, 2048:3072])
        nc.sync.dma_start(out=ch_t[:], in_=ch_d[:])
        gmh = adjt[:, 0:1024].rearrange("p (h q) -> p h q", h=2)
        # column base of mt's stripe in adjt
        stripe_base = [1024, 3072, 4096, 5120, 2048, 6144, 7168, 8192]

        # ---- ma tiles: one [128, 4*4096] tile, 4 rotating column slices.
        # row 0 = shared ones; rows 1-6 = HBM slab (u0,m0,cg0,u1,m1,cg1);
        # rows 7-10 = gathers (ug0,mg0,ug1,mg1).  Rows 11:128 zeroed from
        # DRAM per-slice (slice 0/1 now, 2/3 overlapped later); K=128
        # keeps the PE HAM gating at 8/8.
        ma_big = movap.tile([128, 4 * BLK], h16, tag="mabig", name="mabig")
        ma4 = [ma_big[:, i * BLK:(i + 1) * BLK] for i in range(MOVA_BUFS)]

        def zero_slice(i):
            # engine memset: off the data-rate-starved startup DMA rings
            nc.gpsimd.memset(ma4[i][:, :], 0.0)

        def ones_slice(i):
            nc.gpsimd.dma_start(out=ma4[i][0:1, :],
                                in_=ones_d[i * BLK:(i + 1) * BLK])

        zero_slice(0)
        ones_slice(0)

        gx = [None] * 8
        ma_t = [None] * NBLK

        def emit_g(mt):
            # G psum borrows a vps tile (cols 0:128 used)
            psg = vps.tile([128, 512], f32, tag="v", name=f"psg{mt}")
            base = stripe_base[mt]
            for nt in range(8):
                c0 = base + nt * 128
                nc.tensor.matmul(
                    psg[:, 0:128],
                    adjt[:, c0:c0 + 128],
                    gmh[:, :, nt * 64:(nt + 1) * 64],
                    start=(nt == 0), stop=(nt == 7))
            g = gxp.tile([128, 128], h16, tag=f"gxm{mt}", name=f"gxm{mt}")
            nc.scalar.activation(g[:], psg[:, 0:128], AF.Copy,
                                 bias=0.0, scale=1.0)
            gx[mt] = g

        def emit_slab(blk, eng):
            ma = ma4[blk % MOVA_BUFS]
            # one HBM slab DMA: rows 1:7 = (u0, m0, cg0, u1, m1, cg1)
            eng.dma_start(out=ma[1:7, :], in_=rows_d[blk])

        def emit_gathers(blk, eng):
            mt0, mt1 = blk // 2, 4 + blk // 2
            p0 = (blk % 2) * 64
            ma = ma4[blk % MOVA_BUFS]
            eng.dma_start(out=ma[7:8, :], in_=gx[mt0][p0:p0 + 64, 0:64])
            eng.dma_start(out=ma[8:9, :], in_=gx[mt0][p0:p0 + 64, 64:128])
            eng.dma_start(out=ma[9:10, :], in_=gx[mt1][p0:p0 + 64, 0:64])
            eng.dma_start(out=ma[10:11, :],
                          in_=gx[mt1][p0:p0 + 64, 64:128])
            ma_t[blk] = ma

        def emit_ma(blk):
            # last blocks ride the scalar ring so the swdge ring is empty
            # at teardown (avoids its long DRAIN backlog)
            eng = nc.gpsimd if blk < 6 else nc.scalar
            emit_slab(blk, eng)
            emit_gathers(blk, eng)

        o_t = {}
        wt_t = {}
        rr_t = {}

        def st_a(g):
            blk, j = g // 8, g % 8
            if g == 0:
                emit_g(0)
                emit_g(4)
                # second warmup burst: fills the PE idle window between
                # the G passes and A0 (gather-data wait) so the HAM duty
                # cycle holds 8/8 through the pipeline start
                for wi in range(32):
                    nc.tensor.matmul(po_ps[:, 0:128], scr[:], scr[:],
                                     start=True, stop=True,
                                     skip_group_check=True)
                # per-slice chains: memset (Pool) -> ones -> slab ->
                # gathers, interleaved so slice 0 completes fastest
                emit_slab(0, nc.gpsimd)
                emit_gathers(0, nc.scalar)
                zero_slice(1)
                ones_slice(1)
                emit_slab(1, nc.gpsimd)
                emit_gathers(1, nc.scalar)
                zero_slice(2)
                ones_slice(2)
                zero_slice(3)
                ones_slice(3)
                # remaining adj stripes (1,2,3 then 5,6,7)
                nc.sync.dma_start(out=adjt[:, 3072:6144],
                                  in_=adjs[:, 3072:6144])
            elif g == 1:
                nc.sync.dma_start(out=adjt[:, 6144:9216],
                                  in_=adjs[:, 6144:9216])
            if j == 4 and blk % 2 == 0 and blk // 2 + 1 < 4:
                emit_g(blk // 2 + 1)
                emit_g(4 + blk // 2 + 1)
            if j == 6 and blk + 2 < NBLK:
                emit_ma(blk + 2)
            c0 = j * CHUNK
            vt = vps.tile([128, 512], f32, tag="v", name=f"v{g}")
            nc.tensor.matmul(vt[:], pa_t, ma_t[blk][:, c0:c0 + CHUNK],
                             start=True, stop=True)
            o = op_p.tile([128, CHUNK], h16, tag="o", name=f"o{g}")
            nc.scalar.activation(o[:], vt[:], AF.Prelu,
                                 bias=0.0, scale=1.0, alpha=prelu_a)
            o_t[g] = o

        def st_b(g):
            o = o_t.pop(g)
            k, h = g // 2, g % 2
            if h == 0:
                wt_t[k] = wps.tile([128, 1024], f32, tag="w", name=f"w{k}")
            wt = wt_t[k]
            nc.tensor.matmul(wt[:, h * 512:(h + 1) * 512], pb_t, o[:],
                             start=True, stop=True)
            if h == 1:
                rr = rrp.tile([128, 1024], h16, tag="rr", name=f"rr{k}")
                nc.vector.tensor_scalar(
                    out=rr[:], in0=wt[:],
                    scalar1=bf_t, scalar2=zero_t,
                    op0=ALU.add, op1=ALU.max)
                del wt_t[k]
                rr_t[k] = rr

        po_sb = posbp.tile([128, 512], h16, tag="po_sb", name="po_sb")

        def drain_quarter(t):
            # tile-col t (pairs 16t..16t+16) complete: copy + 32KB DMA
            # on DVE so the ACT prelu chain never hiccups
            nc.vector.tensor_scalar(
                out=po_sb[32 * t:32 * t + 32, :],
                in0=po_ps[32 * t:32 * t + 32, :],
                scalar1=bo2_t[32 * t:32 * t + 32],
                scalar2=zero_t[32 * t:32 * t + 32],
                op0=ALU.add, op1=ALU.add)
            nc.sync.dma_start(out=out_d[t * 16384:(t + 1) * 16384],
                              in_=po_sb[32 * t:32 * t + 32, :])

        def st_c(g):
            k, t = g % 16, g // 16
            rr = rr_t[g // 2]
            nc.tensor.matmul(po_ps[32 * t:32 * t + 32, :], pc_t[k],
                             rr[:, (g % 2) * 512:(g % 2 + 1) * 512],
                             start=(k == 0), stop=(k == 15),
                             tile_position=(0, 32 * t),
                             skip_group_check=True)
            if g % 2 == 1:
                del rr_t[g // 2]
            if k == 15:
                drain_quarter(t)

        # deeper stage lags (B at p-3, C at p-5): one extra pair of slack
        # for the prelu->B and relu->C cross-engine hops
        for p in range(NPAIR + 5):
            if p < NPAIR:
                st_a(p)
            if 3 <= p < NPAIR + 3:
                st_b(p - 3)
            if 5 <= p < NPAIR + 5:
                st_c(p - 5)

    nc.compile()
    return nc


def _get_program(prelu_a):
    key = ("prog", float(prelu_a))
    if key not in _CACHE:
        _CACHE[key] = _build_program(prelu_a)
    return _CACHE[key]


def make_in_maps(x, mask, W_fs, b_fs, W_in, b_in, adj, W_gc, b_gc, W_lo, b_lo,
                 prelu_a, W_ro, b_ro, W_o1, b_o1, W_o2, b_o2):
    x = np.asarray(x, np.float32)
    mask_f = np.asarray(mask, np.float32)
    adj = np.asarray(adj, np.float32)

    folded = _fold_weights(np.asarray(W_fs), np.asarray(b_fs),
                           np.asarray(W_in), np.asarray(b_in),
                           np.asarray(W_gc), np.asarray(b_gc),
                           np.asarray(W_lo), np.asarray(b_lo),
                           float(prelu_a),
                           np.asarray(W_ro), np.asarray(b_ro),
                           np.asarray(W_o1), np.asarray(b_o1),
                           np.asarray(W_o2), np.asarray(b_o2), adj)

    # adj grouped mt-major then packed [um | s0 | s4 | s1,s2,s3 | s5,s6,s7]
    adj_mt = np.ascontiguousarray(
        adj.astype(np.float16).reshape(8, 128, 8, 128).transpose(1, 2, 0, 3)
    ).reshape(128, 8, 1024)
    shared = dict(consts_h=folded["consts_h"],
                  ones=np.ones(4 * BLK, np.float16),
                  zeros=np.zeros((128, 4 * BLK), np.float16))
    cgrep = folded["cgrep"].astype(np.float16)
    u_all = (x[:, 0] - float(np.asarray(b_fs)[0])) * mask_f[:, 0]  # [B, N, T]
    in_maps = []
    for b in range(B):
        m = dict(shared)
        uh = u_all[b].astype(np.float16)          # [N, T]
        mh = mask_f[b, 0].astype(np.float16)
        big = np.empty((128, 9216), np.float16)
        big[:, 0:512] = uh.reshape(8, 128, T).transpose(1, 0, 2).reshape(
            128, 512)
        big[:, 512:1024] = mh.reshape(8, 128, T).transpose(1, 0, 2).reshape(
            128, 512)
        for i, mt in enumerate([0, 4, 1, 2, 3, 5, 6, 7]):
            big[:, 1024 * (i + 1):1024 * (i + 2)] = adj_mt[:, mt]
        m["adjs"] = big
        # per-block 6-row slabs: (u0, m0, cg0, u1, m1, cg1) x 4096
        up = uh.reshape(NT)
        mp = mh.reshape(NT)
        rows = np.empty((NBLK, 6, BLK), np.float16)
        for blk in range(NBLK):
            o0, o1 = blk * BLK, HALF + blk * BLK
            rows[blk, 0] = up[o0:o0 + BLK]
            rows[blk, 1] = mp[o0:o0 + BLK]
            rows[blk, 2] = cgrep[o0:o0 + BLK]
            rows[blk, 3] = up[o1:o1 + BLK]
            rows[blk, 4] = mp[o1:o1 + BLK]
            rows[blk, 5] = cgrep[o1:o1 + BLK]
        m["rows"] = rows
        in_maps.append(m)
    return in_maps, folded["prelu_a"]


def kernel(x, mask, W_fs, b_fs, W_in, b_in, adj, W_gc, b_gc, W_lo, b_lo,
           prelu_a, W_ro, b_ro, W_o1, b_o1, W_o2, b_o2):
    in_maps, a = make_in_maps(x, mask, W_fs, b_fs, W_in, b_in, adj, W_gc,
                              b_gc, W_lo, b_lo, prelu_a, W_ro, b_ro, W_o1,
                              b_o1, W_o2, b_o2)
    nc = _get_program(a)

    from concourse.bass_utils import run_bass_kernel_spmd
    res = run_bass_kernel_spmd(nc, in_maps, list(range(B)))

    out = np.empty((B, C, N, T), np.float32)
    for b in range(B):
        # device row 2g+lane = pair g, lane
        dev = np.asarray(res.results[b]["out"]).reshape(NPAIR, 2, CHUNK)
        flat = np.empty(NT, np.float32)
        fl = flat.reshape(2, NPAIR, CHUNK)     # [lane, chunk, 512]
        fl[0] = dev[:, 0]
        fl[1] = dev[:, 1]
        out[b, 0] = flat.reshape(N, T)
    return out  # fp16 device output upcast to f32 on assignment


# revision 102
# speedup vs baseline: 1.0118x; 1.0118x over previous
"""Bass/Trainium2 kernel for nn_BiGRIL (gnn_message_passing).

Algebraic structure (h == 0, C == 1):
  u    = (x - bfs)*m                    (host-computed input prep)
  z    = W0*x1 + W1*m + b_in            (rank-2 in channels)
  zg   = A^T z  ->  W0*ug + W1*mg + b_in'*cg   with ug = A^T u, mg = A^T m
  v    = PA6^T [1,u,m,ug,mg,cg]         (K=11 matmul; bias via ones-row)
  o    = PReLU(v)                       (ACT Prelu, psum->fp16)
  w    = PB^T o                         (pass B; PB = wro1 (x) W_o1 rank-1)
  rr   = relu(w + bias_f)               (DVE TS over [128,1024] couples)
  out  = W_o2 . rr + b_o2               (pass C; +b_o2 via final copy bias)

Per-pair pipeline (chunk pair g = lanes g, g+64; 512 positions each):
A(p) / B(p-2) / C(p-4) like the reference schedule, with:
 - K=11 moving tiles (shared ones-row carries bias_v; no dead rows, no
   memsets) whose u/m/cg rows stream position-major straight from HBM
 - pass B couples write a [128,1024] 2-bank psum tile; ONE wide DVE
   tensor-scalar relu per couple
 - pass C packs its 2 output rows per pair into ONE accumulated psum
   bank (rows 2g, 2g+1 via 16 stationary column-variants + tile_position
   + start=False accumulation), so the whole output drains with a single
   copy + a single 128KB DMA
 - adj grouped mt-major so G-tiles start as stripes land; G psums borrow
   the vps pool

Sharding: data-parallel over batch (B=8 -> 8 cores), no collectives.
"""

import numpy as np
import sys

sys.path.insert(0, "/opt/trn_rl_repo")

B, C, N, T = 8, 1, 1024, 64
H = 64
NT = N * T          # 65536 per-core output elements
CHUNK = 512
NPAIR = 64          # pairs (g, g+64); chunk c=g is lane0, c=g+64 lane1
HALF = NT // 2      # 32768: lane1 position offset
BLK = 4096          # ma tile columns (64 nodes x 64 steps)
NBLK = 8            # blocks per lane

_CACHE = {}


def _fold_weights(W_fs, b_fs, W_in, b_in, W_gc, b_gc, W_lo, b_lo, prelu_a,
                  W_ro, b_ro, W_o1, b_o1, W_o2, b_o2, adj):
    """Host-side weight folding in float64 for accuracy."""
    f8 = np.float64
    W_in, b_in = W_in.astype(f8), b_in.astype(f8)
    W_gc, b_gc = W_gc.astype(f8), b_gc.astype(f8)
    W_lo, b_lo = W_lo.astype(f8), b_lo.astype(f8)
    W_ro, b_ro = W_ro.astype(f8), b_ro.astype(f8)
    W_o1, b_o1 = W_o1.astype(f8), b_o1.astype(f8)
    W_o2, b_o2 = W_o2.astype(f8), b_o2.astype(f8)

    W0 = W_in[:, 0]           # x1 channel  [64]
    W1 = W_in[:, 1]           # mask channel [64]
    Wlo1 = W_lo[:, :H]
    M1 = Wlo1 @ W_gc[:, :H]
    M2 = Wlo1 @ W_gc[:, H:]
    b_fold = Wlo1 @ b_gc + b_lo

    b_in_p = b_in + W0 * float(b_fs[0])
    PA = np.stack([
        M1 @ W0,
        M1 @ W1,
        M2 @ W0,
        M2 @ W1,
        M2 @ b_in_p,
    ])                                     # [5, 64]  lhsT for pass A
    bias_v = M1 @ b_in_p + b_fold          # [64] -> ones-row of pass A

    w_ro1 = W_ro[0, :H]                    # [64]
    PB = np.outer(w_ro1, W_o1[:, 0])       # [64(h), 64(f)] lhsT for pass B
    bias_f = W_o1[:, 0] * b_ro[0] + b_o1   # [64]

    cg = adj.astype(f8).sum(axis=0)        # [N] column sums of adj
    cgrep = np.repeat(cg, T)               # [(n,t)] layout n*T + t

    a = float(prelu_a)
    assert 0.0 < a < 1.0

    # pass A stationary [128, 128]: row 0 = shared ones stream carrying
    # bias_v for BOTH lanes; rows 1-6 = HBM slab (u0,m0,cg0,u1,m1,cg1);
    # rows 7-10 = gathered (ug0,mg0,ug1,mg1).  Lane0 -> out cols 0:64,
    # lane1 -> cols 64:128.  Rows 11:128 are zero: K stays 128 because
    # K<128 matmuls pin the PE HAM activity gating at 4/8 (half rate).
    paA = np.zeros((128, 128))
    paA[0, 0:64] = bias_v
    paA[0, 64:128] = bias_v
    paA[1, 0:64] = PA[0]        # u0
    paA[2, 0:64] = PA[1]        # m0
    paA[3, 0:64] = PA[4]        # cg0
    paA[4, 64:128] = PA[0]      # u1
    paA[5, 64:128] = PA[1]      # m1
    paA[6, 64:128] = PA[4]      # cg1
    paA[7, 0:64] = PA[2]        # ug0
    paA[8, 0:64] = PA[3]        # mg0
    paA[9, 64:128] = PA[2]      # ug1
    paA[10, 64:128] = PA[3]     # mg1
    # pass B stationary: blockdiag(PB, PB) over packed prelu [128, 512]
    pbB = np.zeros((128, 128))
    pbB[0:64, 0:64] = PB
    pbB[64:128, 64:128] = PB
    # pass C stationaries: 16 variants [128, 32]; variant k places lane0 at
    # col 2k, lane1 at col 2k+1 -> accumulated po-bank row 32t + 2k + lane
    # for pair g with k = g%16, t = g//16
    pc = np.zeros((128, 16 * 32))
    for k in range(16):
        pc[0:64, 32 * k + 2 * k] = W_o2[0]
        pc[64:128, 32 * k + 2 * k + 1] = W_o2[0]

    h16 = np.float16
    fp = np.float32
    ch = np.zeros((128, 776), h16)
    ch[:, 0:128] = paA.astype(h16)
    ch[:, 128:256] = pbB.astype(h16)
    ch[:, 256:768] = pc.astype(h16)
    cf = np.zeros((128, 4), fp)
    cf[0:64, 0] = bias_f           # rr relu bias
    cf[64:128, 0] = bias_f
    cf[:, 1] = float(b_o2[0])      # final copy bias (all rows are outputs)
    # cols 2,3 stay zero: zero-scalar operands for DVE tensor_scalar ops
    ch[:, 768:776] = cf.view(h16)  # fp32 scalars bit-packed as fp16 cols

    return dict(
        consts_h=ch,
        cgrep=cgrep.astype(h16),
        prelu_a=a,
    )


def _build_program(prelu_a):
    import concourse.bass as bass
    import concourse.bacc as bacc
    import concourse.mybir as mybir
    import concourse.tile as tile

    dt = mybir.dt
    f32 = dt.float32
    h16 = dt.float16
    AF = mybir.ActivationFunctionType
    ALU = mybir.AluOpType

    nc = bacc.Bacc("TRN2", target_bir_lowering=False, debug=False,
                   num_devices=B)

    ones_d = nc.dram_tensor("ones", [4 * BLK], h16, kind="ExternalInput")
    zeros_d = nc.dram_tensor("zeros", [128, 4 * BLK], h16,
                             kind="ExternalInput")
    # per-block 6-row slab: (u0, m0, cg0, u1, m1, cg1) x 4096, ma rows 1:7
    rows_d = nc.dram_tensor("rows", [NBLK, 6, BLK], h16,
                            kind="ExternalInput")
    # packed [um | stripe0 | stripe4 | stripes 1,2,3 | stripes 5,6,7]
    adjs = nc.dram_tensor("adjs", [128, 9216], h16, kind="ExternalInput")
    # consts: fp16 cols 0:768 + 8 fp16 cols holding the 4 fp32 scalars
    ch_d = nc.dram_tensor("consts_h", [128, 776], h16, kind="ExternalInput")
    # row-major [128, 512] final tile; row 2g+lane = pair g, lane
    out_d = nc.dram_tensor("out", [NT], h16, kind="ExternalOutput")

    MOVA_BUFS = 4
    from contextlib import ExitStack
    with tile.TileContext(nc) as tc, ExitStack() as ctx:
        const = ctx.enter_context(tc.tile_pool(name="const", bufs=1))
        adjp = ctx.enter_context(tc.tile_pool(name="adjp", bufs=1))
        ump = ctx.enter_context(tc.tile_pool(name="ump", bufs=1))
        gxp = ctx.enter_context(tc.tile_pool(name="gxp", bufs=1))
        movap = ctx.enter_context(tc.tile_pool(name="movap", bufs=1))
        op_p = ctx.enter_context(tc.tile_pool(name="op", bufs=6))
        rrp = ctx.enter_context(tc.tile_pool(name="rrp", bufs=3))
        posbp = ctx.enter_context(tc.tile_pool(name="posbp", bufs=1))
        # PSUM budget (8 banks): vps 3x1 + wps 2x2 + pop 1
        vps = ctx.enter_context(tc.tile_pool(name="vps", bufs=3, space="PSUM"))
        wps = ctx.enter_context(tc.tile_pool(name="wps", bufs=2, space="PSUM"))
        pop = ctx.enter_context(tc.tile_pool(name="pop", bufs=1, space="PSUM"))

        # ---- single consolidated constant load (emitted after the
        # G-critical loads below; nothing needs consts before pair 0) ----
        ch_t = const.tile([128, 776], h16)
        pa_t = ch_t[:, 0:128]
        pb_t = ch_t[:, 128:256]
        pc_t = [ch_t[:, 256 + 32 * k:256 + 32 * (k + 1)] for k in range(16)]
        cf_t = ch_t[:, 768:776].bitcast(f32)   # [128, 4] fp32 view
        bf_t = cf_t[:, 0:1]
        bo2_t = cf_t[:, 1:2]
        zero_t = cf_t[:, 3:4]

        # ---- HAM warmup on a memset scratch: NO DMA dependency, so the
        # PE starts streaming (and ramping clock) at t~0 -------------------
        po_ps = pop.tile([128, 512], f32, tag="po", name="po")
        scr = const.tile([128, 128], h16)
        nc.gpsimd.memset(scr[:, :], 0.0)
        for wi in range(72):
            nc.tensor.matmul(po_ps[:, 0:128], scr[:], scr[:],
                             start=True, stop=True, skip_group_check=True)

        # G inputs ride the sync ring (parallel with the ma-chain on the
        # gpsimd ring).  adjt packs [um | stripe0 | stripe4 | 1,2,3 |
        # 5,6,7] so G0's critical data is ONE DMA.
        adjt = adjp.tile([128, 9216], h16, tag="adjt", name="adjt")
        # G0-critical halves land in parallel on both hw-DGE rings
        nc.sync.dma_start(out=adjt[:, 0:1024], in_=adjs[:, 0:1024])
        nc.scalar.dma_start(out=adjt[:, 1024:2048], in_=adjs[:, 1024:2048])
        nc.sync.dma_start(out=adjt[:, 2048:3072], in_=adjs[:, 2048:3072])
        nc.sync.dma_start(out=ch_t[:], in_=ch_d[:])
        gmh = adjt[:, 0:1024].rearrange("p (h q) -> p h q", h=2)
        # column base of mt's stripe in adjt
        stripe_base = [1024, 3072, 4096, 5120, 2048, 6144, 7168, 8192]

        # ---- ma tiles: one [128, 4*4096] tile, 4 rotating column slices.
        # row 0 = shared ones; rows 1-6 = HBM slab (u0,m0,cg0,u1,m1,cg1);
        # rows 7-10 = gathers (ug0,mg0,ug1,mg1).  Rows 11:128 zeroed from
        # DRAM per-slice (slice 0/1 now, 2/3 overlapped later); K=128
        # keeps the PE HAM gating at 8/8.
        ma_big = movap.tile([128, 4 * BLK], h16, tag="mabig", name="mabig")
        ma4 = [ma_big[:, i * BLK:(i + 1) * BLK] for i in range(MOVA_BUFS)]

        def zero_slice(i):
            # engine memset: off the data-rate-starved startup DMA rings
            nc.gpsimd.memset(ma4[i][:, :], 0.0)

        def ones_slice(i):
            nc.gpsimd.dma_start(out=ma4[i][0:1, :],
                                in_=ones_d[i * BLK:(i + 1) * BLK])

        zero_slice(0)
        ones_slice(0)

        gx = [None] * 8
        ma_t = [None] * NBLK

        def emit_g(mt):
            # G psum borrows a vps tile (cols 0:128 used)
            psg = vps.tile([128, 512], f32, tag="v", name=f"psg{mt}")
            base = stripe_base[mt]
            for nt in range(8):
                c0 = base + nt * 128
                nc.tensor.matmul(
                    psg[:, 0:128],
                    adjt[:, c0:c0 + 128],
                    gmh[:, :, nt * 64:(nt + 1) * 64],
                    start=(nt == 0), stop=(nt == 7))
            g = gxp.tile([128, 128], h16, tag=f"gxm{mt}", name=f"gxm{mt}")
            nc.scalar.activation(g[:], psg[:, 0:128], AF.Copy,
                                 bias=0.0, scale=1.0)
            gx[mt] = g

        def emit_slab(blk, eng):
            ma = ma4[blk % MOVA_BUFS]
            # one HBM slab DMA: rows 1:7 = (u0, m0, cg0, u1, m1, cg1)
            eng.dma_start(out=ma[1:7, :], in_=rows_d[blk])

        def emit_gathers(blk, eng):
            mt0, mt1 = blk // 2, 4 + blk // 2
            p0 = (blk % 2) * 64
            ma = ma4[blk % MOVA_BUFS]
            eng.dma_start(out=ma[7:8, :], in_=gx[mt0][p0:p0 + 64, 0:64])
            eng.dma_start(out=ma[8:9, :], in_=gx[mt0][p0:p0 + 64, 64:128])
            eng.dma_start(out=ma[9:10, :], in_=gx[mt1][p0:p0 + 64, 0:64])
            eng.dma_start(out=ma[10:11, :],
                          in_=gx[mt1][p0:p0 + 64, 64:128])
            ma_t[blk] = ma

        def emit_ma(blk):
            # last blocks ride the scalar ring so the swdge ring is empty
            # at teardown (avoids its long DRAIN backlog)
            eng = nc.gpsimd if blk < 6 else nc.scalar
            emit_slab(blk, eng)
            emit_gathers(blk, eng)

        o_t = {}
        wt_t = {}
        rr_t = {}

        def st_a(g):
            blk, j = g // 8, g % 8
            if g == 0:
                emit_g(0)
                emit_g(4)
                # second warmup burst: fills the PE idle window between
                # the G passes and A0 (gather-data wait) so the HAM duty
                # cycle holds 8/8 through the pipeline start
                for wi in range(28):
                    nc.tensor.matmul(po_ps[:, 0:128], scr[:], scr[:],
                                     start=True, stop=True,
                                     skip_group_check=True)
                # per-slice chains: memset (Pool) -> ones -> slab ->
                # gathers, interleaved so slice 0 completes fastest
                emit_slab(0, nc.gpsimd)
                emit_gathers(0, nc.scalar)
                zero_slice(1)
                ones_slice(1)
                emit_slab(1, nc.gpsimd)
                emit_gathers(1, nc.scalar)
                zero_slice(2)
                ones_slice(2)
                zero_slice(3)
                ones_slice(3)
                # remaining adj stripes (1,2,3 then 5,6,7)
                nc.sync.dma_start(out=adjt[:, 3072:6144],
                                  in_=adjs[:, 3072:6144])
            elif g == 1:
                nc.sync.dma_start(out=adjt[:, 6144:9216],
                                  in_=adjs[:, 6144:9216])
            if j == 4 and blk % 2 == 0 and blk // 2 + 1 < 4:
                emit_g(blk // 2 + 1)
                emit_g(4 + blk // 2 + 1)
            if j == 6 and blk + 2 < NBLK:
                emit_ma(blk + 2)
            c0 = j * CHUNK
            vt = vps.tile([128, 512], f32, tag="v", name=f"v{g}")
            nc.tensor.matmul(vt[:], pa_t, ma_t[blk][:, c0:c0 + CHUNK],
                             start=True, stop=True)
            o = op_p.tile([128, CHUNK], h16, tag="o", name=f"o{g}")
            nc.scalar.activation(o[:], vt[:], AF.Prelu,
                                 bias=0.0, scale=1.0, alpha=prelu_a)
            o_t[g] = o

        def st_b(g):
            o = o_t.pop(g)
            k, h = g // 2, g % 2
            if h == 0:
                wt_t[k] = wps.tile([128, 1024], f32, tag="w", name=f"w{k}")
            wt = wt_t[k]
            nc.tensor.matmul(wt[:, h * 512:(h + 1) * 512], pb_t, o[:],
                             start=True, stop=True)
            if h == 1:
                rr = rrp.tile([128, 1024], h16, tag="rr", name=f"rr{k}")
                nc.vector.tensor_scalar(
                    out=rr[:], in0=wt[:],
                    scalar1=bf_t, scalar2=zero_t,
                    op0=ALU.add, op1=ALU.max)
                del wt_t[k]
                rr_t[k] = rr

        po_sb = posbp.tile([128, 512], h16, tag="po_sb", name="po_sb")

        def drain_quarter(t):
            # tile-col t (pairs 16t..16t+16) complete: copy + 32KB DMA
            # on DVE so the ACT prelu chain never hiccups
            nc.vector.tensor_scalar(
                out=po_sb[32 * t:32 * t + 32, :],
                in0=po_ps[32 * t:32 * t + 32, :],
                scalar1=bo2_t[32 * t:32 * t + 32],
                scalar2=zero_t[32 * t:32 * t + 32],
                op0=ALU.add, op1=ALU.add)
            nc.sync.dma_start(out=out_d[t * 16384:(t + 1) * 16384],
                              in_=po_sb[32 * t:32 * t + 32, :])

        def st_c(g):
            k, t = g % 16, g // 16
            rr = rr_t[g // 2]
            nc.tensor.matmul(po_ps[32 * t:32 * t + 32, :], pc_t[k],
                             rr[:, (g % 2) * 512:(g % 2 + 1) * 512],
                             start=(k == 0), stop=(k == 15),
                             tile_position=(0, 32 * t),
                             skip_group_check=True)
            if g % 2 == 1:
                del rr_t[g // 2]
            if k == 15:
                drain_quarter(t)

        # deeper stage lags (B at p-3, C at p-5): one extra pair of slack
        # for the prelu->B and relu->C cross-engine hops
        for p in range(NPAIR + 5):
            if p < NPAIR:
                st_a(p)
            if 3 <= p < NPAIR + 3:
                st_b(p - 3)
            if 5 <= p < NPAIR + 5:
                st_c(p - 5)

    nc.compile()
    return nc


def _get_program(prelu_a):
    key = ("prog", float(prelu_a))
    if key not in _CACHE:
        _CACHE[key] = _build_program(prelu_a)
    return _CACHE[key]


def make_in_maps(x, mask, W_fs, b_fs, W_in, b_in, adj, W_gc, b_gc, W_lo, b_lo,
                 prelu_a, W_ro, b_ro, W_o1, b_o1, W_o2, b_o2):
    x = np.asarray(x, np.float32)
    mask_f = np.asarray(mask, np.float32)
    adj = np.asarray(adj, np.float32)

    folded = _fold_weights(np.asarray(W_fs), np.asarray(b_fs),
                           np.asarray(W_in), np.asarray(b_in),
                           np.asarray(W_gc), np.asarray(b_gc),
                           np.asarray(W_lo), np.asarray(b_lo),
                           float(prelu_a),
                           np.asarray(W_ro), np.asarray(b_ro),
                           np.asarray(W_o1), np.asarray(b_o1),
                           np.asarray(W_o2), np.asarray(b_o2), adj)

    # adj grouped mt-major then packed [um | s0 | s4 | s1,s2,s3 | s5,s6,s7]
    adj_mt = np.ascontiguousarray(
        adj.astype(np.float16).reshape(8, 128, 8, 128).transpose(1, 2, 0, 3)
    ).reshape(128, 8, 1024)
    shared = dict(consts_h=folded["consts_h"],
                  ones=np.ones(4 * BLK, np.float16),
                  zeros=np.zeros((128, 4 * BLK), np.float16))
    cgrep = folded["cgrep"].astype(np.float16)
    u_all = (x[:, 0] - float(np.asarray(b_fs)[0])) * mask_f[:, 0]  # [B, N, T]
    in_maps = []
    for b in range(B):
        m = dict(shared)
        uh = u_all[b].astype(np.float16)          # [N, T]
        mh = mask_f[b, 0].astype(np.float16)
        big = np.empty((128, 9216), np.float16)
        big[:, 0:512] = uh.reshape(8, 128, T).transpose(1, 0, 2).reshape(
            128, 512)
        big[:, 512:1024] = mh.reshape(8, 128, T).transpose(1, 0, 2).reshape(
            128, 512)
        for i, mt in enumerate([0, 4, 1, 2, 3, 5, 6, 7]):
            big[:, 1024 * (i + 1):1024 * (i + 2)] = adj_mt[:, mt]
        m["adjs"] = big
        # per-block 6-row slabs: (u0, m0, cg0, u1, m1, cg1) x 4096
        up = uh.reshape(NT)
        mp = mh.reshape(NT)
        rows = np.empty((NBLK, 6, BLK), np.float16)
        for blk in range(NBLK):
            o0, o1 = blk * BLK, HALF + blk * BLK
            rows[blk, 0] = up[o0:o0 + BLK]
            rows[blk, 1] = mp[o0:o0 + BLK]
            rows[blk, 2] = cgrep[o0:o0 + BLK]
            rows[blk, 3] = up[o1:o1 + BLK]
            rows[blk, 4] = mp[o1:o1 + BLK]
            rows[blk, 5] = cgrep[o1:o1 + BLK]
        m["rows"] = rows
        in_maps.append(m)
    return in_maps, folded["prelu_a"]


def kernel(x, mask, W_fs, b_fs, W_in, b_in, adj, W_gc, b_gc, W_lo, b_lo,
           prelu_a, W_ro, b_ro, W_o1, b_o1, W_o2, b_o2):
    in_maps, a = make_in_maps(x, mask, W_fs, b_fs, W_in, b_in, adj, W_gc,
                              b_gc, W_lo, b_lo, prelu_a, W_ro, b_ro, W_o1,
                              b_o1, W_o2, b_o2)
    nc = _get_program(a)

    from concourse.bass_utils import run_bass_kernel_spmd
    res = run_bass_kernel_spmd(nc, in_maps, list(range(B)))

    out = np.empty((B, C, N, T), np.float32)
    for b in range(B):
        # device row 2g+lane = pair g, lane
        dev = np.asarray(res.results[b]["out"]).reshape(NPAIR, 2, CHUNK)
        flat = np.empty(NT, np.float32)
        fl = flat.reshape(2, NPAIR, CHUNK)     # [lane, chunk, 512]
        fl[0] = dev[:, 0]
        fl[1] = dev[:, 1]
        out[b, 0] = flat.reshape(N, T)
    return out  # fp16 device output upcast to f32 on assignment
